# revision 85
# baseline (speedup 1.0000x reference)
"""DualEEG connectivity features on 8 Trainium2 NeuronCores (Bass/Tile).

Sharding: core c -> batch c//2, band-half c%2 (bands {0,1,2} / {3,4,5}).
One SPMD program; per-core variation lives in input tensor content.

v3 (vs baseline):
  - all DFT matmuls in bf16 (fwd + inv): 4x fewer PE cycles than fp32
  - inv DFT emits t-halves into [128,1024] psum; station col-swap for the
    second half so b folds partition-aligned into a [128,1024] tile
    (128-partition folding halves every elementwise op downstream)
  - phases: recip/mult/arctan/sign/correction on folded tiles; p written
    directly as bf16 (hi-only); stats accumulators ride existing ops
  - grams: transpose p (not cos/sin), then Sin / Sin(+pi/2) in t-major
    layout; tcorr/pcorr operands transposed as one [b|pw] tile
  - pair stage: d and pw fields via K=64 bf16 matmuls into [128,512]
    psum tiles; per tile three reduction passes (sum|d|, sign-count,
    sum pw*[d>0]) spread over Act/DVE/Pool by a deterministic greedy
    schedule mirrored host-side
  - software pipelining: iteration u emits inv(u) -> pair(u-1) ->
    phases(u) -> transposes/grams(u-1) so each engine's queue leads
    with ready work
  - wpli numerator via 2*sum(pw*[d>0]) - sum(pw); pdiff via sum|d|
"""
import numpy as np
import ml_dtypes

import concourse.bass as bass
import concourse.mybir as mybir
import concourse.tile as tile
from concourse.bass_utils import run_bass_kernel_spmd

F32 = mybir.dt.float32
BF16 = mybir.dt.bfloat16
AF = mybir.ActivationFunctionType
ALU = mybir.AluOpType


def legalize_waits(nc, max_waits=1):
    """This walrus build rejects instructions with >1 semaphore wait.
    Hoist extra waits onto NoOps inserted before the instruction on the
    same engine (engine program order runs them first)."""
    ctr = 0
    n_fixed = 0
    for bb in nc.main_func.blocks:
        out = []
        changed = False
        for ins in bb.instructions:
            si = ins.sync_info
            if si is not None and si.on_wait and len(si.on_wait) > max_waits:
                waits = list(si.on_wait)
                for w in waits[:-max_waits]:
                    ctr += 1
                    nop = mybir.InstNoOp(name=f"waitfix-{ctr}", ins=[], outs=[])
                    nop.engine = ins.engine
                    nop.sync_info = mybir.SyncInfo(on_wait=[w], on_update=[])
                    out.append(nop)
                ins.sync_info = mybir.SyncInfo(
                    on_wait=waits[-max_waits:], on_update=si.on_update)
                n_fixed += 1
                changed = True
            out.append(ins)
        if changed:
            try:
                bb.instructions = out
            except Exception:
                li = bb.instructions
                li.clear()
                li.extend(out)
    return n_fixed


FS = 256
T = 2048
HT = T // 2               # 1024, t-half length
TW = 512                  # pair tile t-width
C = 32
EPS = 1e-8
BANDS = [(0.5, 45.0), (0.5, 4.0), (4.0, 8.0), (8.0, 13.0), (13.0, 30.0),
         (30.0, 45.0)]
NRFFT = T // 2 + 1        # 1025

NSLOT = 9                 # inv-DFT slots (64 freqs each)
U_SLOTS = [(0, 6), (6, 8), (8, 9)]  # unit -> slot range
RL = NSLOT * 64           # padded row-list length = 576
NTC = T // 128            # 16 t-chunks
NFC = 5                   # fwd f-chunks: 4x128 + 1x64
NPT = 8                   # pair row-tiles (1024 pairs / 128)
NPH = NPT * (T // TW)     # 32 pair tiles per unit
PI = float(np.float32(np.pi))
HPI = float(np.float32(np.pi / 2))

UNIT_BANDS = [[0, 1, 2], [4, 5, 3]]

# ---------------------------------------------------------------------------
# pair-pass tile configs (GPSIMD cannot read PSUM, so Pool only ever sees
# bf16 SBUF copies):
#  S: Act copies dps->d16 bf16; Pool does |d| and count from d16; DVE does
#     the pw-weighted pass from psum.     acc_c = count, acc_w = sum pw*[d>0]
#  P: Act Abs + Act Sign from psum; DVE pw-pass.
#     acc_c = sum sign, acc_w = sum pw*[d>0]
#  Q: Act Sign -> sgn16 (+sum sign); DVE ttr(dps*sgn16) -> sum|d| and
#     ttr(pps*sgn16) -> sum pw*sign.     acc_c = sum sign, acc_w = sum pw*sgn
CFG_COST = {
    "P": {"act": 1.54, "dve": 0.64},
    "Q": {"act": 0.77, "dve": 1.28},
}
BASE_LOAD = {"act": 12.0, "dve": 9.9}


def build_pass_schedule():
    """One config per pair-row group r (all 4 t-quarters share it, so the
    host-side wpli identity 2*sum(pw*[d>0]) - sum(pw) stays exact per pair).
    """
    load = dict(BASE_LOAD)
    sched = []
    for r in range(NPT):
        best = min(CFG_COST, key=lambda c: max(
            load[e] + 4 * CFG_COST[c][e] for e in load))
        for e in load:
            load[e] += 4 * CFG_COST[best][e]
        sched.append(best)
    return sched


PASS_SCHED = build_pass_schedule()  # indexed by r = ht // 4


def band_rows(bi):
    freqs = np.fft.rfftfreq(T, d=1.0 / FS)
    lo, hi = BANDS[bi]
    return np.where((freqs >= lo) & (freqs <= hi))[0]


def rowlist(half):
    rows = np.full(RL, -1, np.int64)
    for u, bi in enumerate(UNIT_BANDS[half]):
        s0, _s1 = U_SLOTS[u]
        r = band_rows(bi)
        rows[s0 * 64: s0 * 64 + len(r)] = r
    return rows


def _bf16(x):
    return np.asarray(x, np.float32).astype(ml_dtypes.bfloat16)


def host_tables(half):
    """fwd wtc/wts [128, NFC*NTC*128] and inv tbl [128, NSLOT*T], bf16."""
    rows = rowlist(half)
    t = np.arange(T)
    valid = rows >= 0
    rr = np.where(valid, rows, 0)
    ang = 2 * np.pi * np.outer(rr, t) / T          # [RL, T]
    cos_ft = (np.cos(ang) * valid[:, None]).astype(np.float32)
    sin_ft = (np.sin(ang) * valid[:, None]).astype(np.float32)
    wtc = np.zeros((128, NFC * NTC * 128), np.float32)
    wts = np.zeros((128, NFC * NTC * 128), np.float32)
    for fc in range(NFC):
        nf = 128 if fc < 4 else 64
        for tc in range(NTC):
            c0 = (fc * NTC + tc) * 128
            wtc[:, c0:c0 + nf] = cos_ft[fc * 128:fc * 128 + nf,
                                        tc * 128:(tc + 1) * 128].T
            wts[:, c0:c0 + nf] = -sin_ft[fc * 128:fc * 128 + nf,
                                         tc * 128:(tc + 1) * 128].T
    # inv table: slot rows 0:64 = cos(f_r, t), 64:128 = sin(f_r, t)
    tbl = np.zeros((128, NSLOT * T), np.float32)
    for s in range(NSLOT):
        tbl[0:64, s * T:(s + 1) * T] = cos_ft[s * 64:(s + 1) * 64]
        tbl[64:128, s * T:(s + 1) * T] = sin_ft[s * 64:(s + 1) * 64]
    return _bf16(wtc), _bf16(wts), _bf16(tbl)


def host_stations():
    """dst [128, NPT*128]: +-1 at hi rows (i, 32+j) and lo rows (64+i, 96+j)
    for the hi/lo bf16 phase split; pwst (+1,+1) in rows 64:128 of a
    [128, .] tensor so its base partition matches the pw rows of gp."""
    dst = np.zeros((128, NPT * 128), np.float32)
    pwst = np.zeros((128, NPT * 128), np.float32)
    for r in range(NPT):
        for m in range(128):
            P = r * 128 + m
            i, j = P // 32, P % 32
            col = r * 128 + m
            dst[i, col] = 1.0
            dst[32 + j, col] = -1.0
            dst[64 + i, col] = 1.0
            dst[96 + j, col] = -1.0
            pwst[64 + i, col] = 1.0
            pwst[64 + 32 + j, col] = 1.0
    return _bf16(dst), _bf16(pwst)


def register_const(nc, value, dtype=F32):
    key = (dtype, float(value))
    if key not in nc.const_aps.aps:
        tns = nc.alloc_sbuf_tensor(f"const-{dtype.name}-{value}", [128, 1], dtype)
        nc.gpsimd.memset(tns.ap(), value)
        nc.const_aps.aps[key] = tns.ap()
    return nc.const_aps.aps[key]


def build_nc():
    nc = bass.Bass()
    for v in (PI, HPI, -1.0, 2.0, 0.5):
        register_const(nc, v)

    xt = nc.dram_tensor("xt", [128, NTC * 64], BF16, kind="ExternalInput")
    wtcD = nc.dram_tensor("wtc", [128, NFC * NTC * 128], BF16,
                          kind="ExternalInput")
    wtsD = nc.dram_tensor("wts", [128, NFC * NTC * 128], BF16,
                          kind="ExternalInput")
    tblD = nc.dram_tensor("tbl", [128, NSLOT * T], BF16, kind="ExternalInput")
    dstD = nc.dram_tensor("dst", [128, NPT * 128], BF16, kind="ExternalInput")
    pwstD = nc.dram_tensor("pwst", [128, NPT * 128], BF16,
                           kind="ExternalInput")
    identD = nc.dram_tensor("ident", [128, 128], BF16, kind="ExternalInput")

    o_gram = nc.dram_tensor("o_gram", [32, 3 * 160], F32, kind="ExternalOutput")
    o_aa = nc.dram_tensor("o_aa", [128, 3 * NPH], F32, kind="ExternalOutput")
    o_ac = nc.dram_tensor("o_ac", [128, 3 * NPH], F32, kind="ExternalOutput")
    o_aw = nc.dram_tensor("o_aw", [128, 3 * NPH], F32, kind="ExternalOutput")
    o_st = nc.dram_tensor("o_st", [128, 3 * 5], F32, kind="ExternalOutput")

    with tile.TileContext(nc) as tc:
        with (
            tc.tile_pool(name="cst", bufs=1) as cst,
            tc.tile_pool(name="fstream", bufs=3) as fstream,
            tc.tile_pool(name="unit", bufs=2) as up,
            tc.tile_pool(name="scr", bufs=2) as scr,
            tc.tile_pool(name="jnk", bufs=3) as jp,
            tc.tile_pool(name="acc", bufs=1) as accp,
        ):
            # ------- constants in (stations/ident deferred past weights so
            # the first fwd weight chunk isn't queued behind them) -------
            xt_sb = cst.tile([128, NTC * 64], BF16, tag="xt")
            nc.sync.dma_start(xt_sb[:], xt[:])
            dst_sb = cst.tile([128, NPT * 128], BF16, tag="dst")
            pwst_sb = cst.tile([128, NPT * 128], BF16, tag="pwst")
            ident_sb = cst.tile([128, 128], BF16, tag="ident")

            # prefetch unit-0 th0 inv tables on idle queues so the first
            # inverse DFT isn't stuck behind the fwd weight stream on SP
            pre_tb = {}

            def prefetch_tb(slots, eng):
                for s in slots:
                    for th in range(2):
                        t0 = cst.tile([128, HT], BF16, tag=f"pre{s}_{th}")
                        eng.dma_start(
                            t0[:], tblD[:, s * T + th * HT:
                                        s * T + (th + 1) * HT])
                        pre_tb[(s, th)] = t0

            prefetch_tb(range(6), nc.gpsimd)

            # ------- forward DFT (bf16) -------
            # xsr per fc: [re|im|re] (cols 0:128 = normal view, 64:192 =
            # swapped [im|re] view); xnr per fc: [imn|re|imn].
            xsr = cst.tile([128, NFC * 192], BF16, tag="xsr")
            xnr = cst.tile([128, NFC * 192], BF16, tag="xnr")
            psum_pools = tc.tile_pool(name="invp", bufs=1, space="PSUM")
            ivp = psum_pools.__enter__()
            _fwdp = tc.tile_pool(name="fwdp", bufs=2, space="PSUM")
            fp = _fwdp.__enter__()

            def emit_fwd_fc(fc):
                    nf = 128 if fc < 4 else 64
                    psr = fp.tile([128, 64], F32, tag="psr")
                    psi = fp.tile([128, 64], F32, tag="psi")
                    wcs = fstream.tile([128, NTC * 128], BF16, tag="wcs")
                    wss = fstream.tile([128, NTC * 128], BF16, tag="wss")
                    fb = fc * NTC * 128
                    nc.sync.dma_start(wcs[:], wtcD[:, fb:fb + NTC * 128])
                    nc.scalar.dma_start(wss[:], wtsD[:, fb:fb + NTC * 128])
                    for tcb in range(NTC):
                        first, last = tcb == 0, tcb == NTC - 1
                        xtb = xt_sb[:, tcb * 64:(tcb + 1) * 64]
                        nc.tensor.matmul(
                            psr[0:nf, :],
                            lhsT=wcs[:, tcb * 128:tcb * 128 + nf],
                            rhs=xtb, start=first, stop=last)
                        nc.tensor.matmul(
                            psi[0:nf, :],
                            lhsT=wss[:, tcb * 128:tcb * 128 + nf],
                            rhs=xtb, start=first, stop=last)
                    xb = fc * 192
                    nc.scalar.copy(xsr[0:nf, xb:xb + 64], psr[0:nf, :])
                    nc.scalar.copy(xsr[0:nf, xb + 64:xb + 128], psi[0:nf, :])
                    nc.scalar.copy(xsr[0:nf, xb + 128:xb + 192],
                                   psr[0:nf, :])
                    nc.vector.tensor_scalar_mul(
                        xnr[0:nf, xb:xb + 64], psi[0:nf, :], -1.0)
                    nc.vector.tensor_copy(xnr[0:nf, xb + 64:xb + 128],
                                          psr[0:nf, :])
                    nc.vector.tensor_scalar_mul(
                        xnr[0:nf, xb + 128:xb + 192], psi[0:nf, :], -1.0)

            # ------- station assembly (bf16) -------
            # slot block [128, 256]: cols 0:128 normal (th0: [re|im] over
            # [imn|re] -> b|him), cols 128:256 swapped (th1: him|b) so th1's
            # b/him land on opposite psum partitions (aligned folds).
            stk = cst.tile([128, NSLOT * 256], BF16, tag="stk")
            dma_engs = [nc.sync, nc.scalar, nc.gpsimd]

            def emit_station(s):
                fcb, sub = divmod(s, 2)
                rsl = slice(sub * 64, (sub + 1) * 64)
                xb = fcb * 192
                eng = dma_engs[s % 3]
                c0 = s * 256
                eng.dma_start(stk[0:64, c0:c0 + 128],
                              xsr[rsl, xb:xb + 128])
                eng.dma_start(stk[64:128, c0:c0 + 128],
                              xnr[rsl, xb:xb + 128])
                eng.dma_start(stk[0:64, c0 + 128:c0 + 256],
                              xsr[rsl, xb + 64:xb + 192])
                eng.dma_start(stk[64:128, c0 + 128:c0 + 256],
                              xnr[rsl, xb + 64:xb + 192])

            # ------- accumulators -------
            acc_a = accp.tile([128, 3 * NPH], F32, tag="aa")
            acc_c = accp.tile([128, 3 * NPH], F32, tag="ac")
            acc_w = accp.tile([128, 3 * NPH], F32, tag="aw")
            stats = accp.tile([128, 3 * 5], F32, tag="stats")
            gram_sb = accp.tile([32, 3 * 160], F32, tag="gram")


            # per-unit state carried across pipeline iterations
            ust = [None, None, None]

            def emit_inv_and_folds(u):
                s0, s1 = U_SLOTS[u]
                sc = u * 5
                b2 = up.tile([128, HT], F32, tag="b2")
                hs = up.tile([128, HT], F32, tag="hs")
                h2 = up.tile([128, HT], F32, tag="h2")
                for th in range(2):
                    pst = ivp.tile([128, HT], F32, tag="pst")
                    for s in range(s0, s1):
                        tb = pre_tb[(s, th)]
                        lhs = stk[:, s * 256 + th * 128:
                                  s * 256 + th * 128 + 128]
                        for ns in range(2):
                            sl = slice(ns * 512, (ns + 1) * 512)
                            nc.tensor.matmul(
                                pst[:, sl], lhsT=lhs, rhs=tb[:, sl],
                                start=(s == s0), stop=(s == s1 - 1))
                    if th == 0:
                        nc.scalar.activation(
                            b2[0:64, :], pst[0:64, :], AF.Copy,
                            accum_out=stats[0:64, sc:sc + 1])
                        nc.vector.tensor_copy(hs[64:128, :], pst[64:128, :])
                    else:
                        nc.scalar.activation(
                            b2[64:128, :], pst[64:128, :], AF.Copy,
                            accum_out=stats[64:128, sc:sc + 1])
                        nc.vector.tensor_copy(hs[0:64, :], pst[0:64, :])
                nc.sync.dma_start(h2[0:64, :], hs[64:128, :])
                nc.sync.dma_start(h2[64:128, :], hs[0:64, :])
                ust[u] = {"b2": b2, "h2": h2}

            def emit_phases(u):
                st = ust[u]
                b2, h2 = st["b2"], st["h2"]
                sc = u * 5
                rb = scr.tile([128, HT], F32, tag="sA")
                nc.vector.reciprocal(rb[:], b2[:])
                ratio = scr.tile([128, HT], F32, tag="sB")
                nc.gpsimd.tensor_tensor(ratio[:], h2[:], rb[:], ALU.mult)
                at = scr.tile([128, HT], F32, tag="sA")
                nc.scalar.activation(at[:], ratio[:], AF.Arctan)
                sgnh = scr.tile([128, HT], BF16, tag="sC")
                nc.scalar.activation(sgnh[:], h2[:], AF.Sign)
                corr = scr.tile([128, HT], BF16, tag="sD")
                nc.vector.scalar_tensor_tensor(
                    out=corr[:], in0=b2[:], scalar=0.0, in1=sgnh[:],
                    op0=ALU.is_lt, op1=ALU.mult)
                pf = scr.tile([128, HT], F32, tag="pf")
                nc.vector.scalar_tensor_tensor(
                    out=pf[:], in0=corr[:], scalar=PI, op0=ALU.mult,
                    op1=ALU.add, in1=at[:])
                p2b = up.tile([128, HT], BF16, tag="p2b")
                nc.scalar.copy(p2b[:], pf[:])
                plo = up.tile([128, HT], BF16, tag="plo")
                nc.gpsimd.tensor_tensor(plo[:], pf[:], p2b[:], ALU.subtract)
                sqb = scr.tile([128, HT], F32, tag="sB")
                nc.scalar.activation(sqb[:], b2[:], AF.Square,
                                     accum_out=stats[:, sc + 2:sc + 3])
                sqh = scr.tile([128, HT], F32, tag="sE")
                nc.gpsimd.tensor_tensor(sqh[:], h2[:], h2[:], ALU.mult)
                pwf = scr.tile([128, HT], F32, tag="sA")
                nc.vector.scalar_tensor_tensor(
                    out=pwf[:], in0=sqb[:], scalar=1.0, in1=sqh[:],
                    op0=ALU.mult, op1=ALU.add,
                    accum_out=stats[:, sc + 3:sc + 4])
                spw2j = scr.tile([128, HT], BF16, tag="sD")
                nc.vector.scalar_tensor_tensor(
                    out=spw2j[:], in0=pwf[:], scalar=1.0, in1=pwf[:],
                    op0=ALU.mult, op1=ALU.mult,
                    accum_out=stats[:, sc + 4:sc + 5])
                b16 = scr.tile([128, HT], BF16, tag="sF")
                nc.gpsimd.tensor_copy(b16[:], b2[:])
                pw16 = scr.tile([128, HT], BF16, tag="sG")
                nc.gpsimd.tensor_copy(pw16[:], pwf[:])
                # unfold via SBUF->SBUF DMAs into matmul operand layouts
                # phl rows 0:64 = p-hi (ch), rows 64:128 = p-lo (ch)
                phl = up.tile([128, T], BF16, tag="phl")
                nc.sync.dma_start(phl[0:64, 0:HT], p2b[0:64, :])
                nc.scalar.dma_start(phl[0:64, HT:T], p2b[64:128, :])
                nc.sync.dma_start(phl[64:128, 0:HT], plo[0:64, :])
                nc.scalar.dma_start(phl[64:128, HT:T], plo[64:128, :])
                gp = up.tile([128, T], BF16, tag="gp")
                nc.sync.dma_start(gp[0:64, 0:HT], b16[0:64, :])
                nc.scalar.dma_start(gp[0:64, HT:T], b16[64:128, :])
                nc.gpsimd.dma_start(gp[64:128, 0:HT], pw16[0:64, :])
                nc.gpsimd.dma_start(gp[64:128, HT:T], pw16[64:128, :])
                st["phl"], st["gp"] = phl, gp

            def emit_pair(u):
                st = ust[u]
                phl, gp = st["phl"], st["gp"]
                for ht in range(NPH):
                    r, q = divmod(ht, 4)
                    msl = slice(q * TW, (q + 1) * TW)
                    dps = dpp.tile([128, TW], F32, tag="dps")
                    nc.tensor.matmul(
                        dps[:], lhsT=dst_sb[:, r * 128:(r + 1) * 128],
                        rhs=phl[:, msl], start=True, stop=True)
                    pps = ppp.tile([128, TW], F32, tag="pps")
                    nc.tensor.matmul(
                        pps[:], lhsT=pwst_sb[64:128, r * 128:(r + 1) * 128],
                        rhs=gp[64:128, msl], start=True, stop=True)
                    k = u * NPH + ht
                    cfg = PASS_SCHED[r]
                    if cfg == "S":
                        d16 = jp.tile([128, TW], BF16, tag="d16")
                        nc.scalar.copy(d16[:], dps[:])
                        ja = jp.tile([128, TW], BF16, tag="ja")
                        nc.gpsimd.tensor_scalar(
                            out=ja[:], in0=d16[:], scalar1=0.0,
                            scalar2=0.0, op0=ALU.abs_max, op1=ALU.add,
                            accum_out=acc_a[:, k:k + 1])
                        jc = jp.tile([128, TW], BF16, tag="jc")
                        nc.gpsimd.tensor_scalar(
                            out=jc[:], in0=d16[:], scalar1=0.0,
                            scalar2=0.0, op0=ALU.is_gt, op1=ALU.add,
                            accum_out=acc_c[:, k:k + 1])
                        jw = jp.tile([128, TW], BF16, tag="jw")
                        nc.vector.scalar_tensor_tensor(
                            out=jw[:], in0=d16[:], scalar=0.0, in1=pps[:],
                            op0=ALU.is_gt, op1=ALU.mult,
                            accum_out=acc_w[:, k:k + 1])
                    elif cfg == "P":
                        ja = jp.tile([128, TW], BF16, tag="ja")
                        nc.scalar.activation(
                            ja[:], dps[:], AF.Abs,
                            accum_out=acc_a[:, k:k + 1])
                        sgn16 = jp.tile([128, TW], BF16, tag="jc")
                        nc.scalar.activation(
                            sgn16[:], dps[:], AF.Sign,
                            accum_out=acc_c[:, k:k + 1])
                        jw = jp.tile([128, TW], BF16, tag="jw")
                        nc.vector.scalar_tensor_tensor(
                            out=jw[:], in0=pps[:], scalar=1.0,
                            in1=sgn16[:], op0=ALU.mult, op1=ALU.mult,
                            accum_out=acc_w[:, k:k + 1])
                    else:  # Q
                        sgn16 = jp.tile([128, TW], BF16, tag="d16")
                        nc.scalar.activation(
                            sgn16[:], dps[:], AF.Sign,
                            accum_out=acc_c[:, k:k + 1])
                        ja = jp.tile([128, TW], BF16, tag="ja")
                        nc.vector.scalar_tensor_tensor(
                            out=ja[:], in0=dps[:], scalar=1.0,
                            in1=sgn16[:], op0=ALU.mult, op1=ALU.mult,
                            accum_out=acc_a[:, k:k + 1])
                        jw = jp.tile([128, TW], BF16, tag="jw")
                        nc.vector.scalar_tensor_tensor(
                            out=jw[:], in0=pps[:], scalar=1.0,
                            in1=sgn16[:], op0=ALU.mult, op1=ALU.mult,
                            accum_out=acc_w[:, k:k + 1])

            def emit_grams(u):
                st = ust[u]
                phl, gp = st["phl"], st["gp"]
                pT = scr.tile([128, NTC * 64], BF16, tag="pT")
                gpT = scr.tile([128, NTC * 128], BF16, tag="gpT")
                for bidx in range(2):
                    ptp = tpp.tile([128, 512], BF16, tag="ptp")
                    for kk in range(8):
                        tau = bidx * 8 + kk
                        nc.tensor.transpose(
                            ptp[:, kk * 64:(kk + 1) * 64],
                            phl[0:64, tau * 128:(tau + 1) * 128],
                            ident_sb[0:64, 0:64])
                    nc.vector.tensor_copy(
                        pT[:, bidx * 512:(bidx + 1) * 512], ptp[:])
                for bidx in range(4):
                    gtp = tpp.tile([128, 512], BF16, tag="ptp")
                    for kk in range(4):
                        tau = bidx * 4 + kk
                        nc.tensor.transpose(
                            gtp[:, kk * 128:(kk + 1) * 128],
                            gp[:, tau * 128:(tau + 1) * 128],
                            ident_sb[:])
                    if bidx % 2 == 0:
                        nc.vector.tensor_copy(
                            gpT[:, bidx * 512:(bidx + 1) * 512], gtp[:])
                    else:
                        nc.scalar.copy(
                            gpT[:, bidx * 512:(bidx + 1) * 512], gtp[:])
                csT_s = scr.tile([128, NTC * 64], BF16, tag="csTs")
                nc.scalar.activation(csT_s[:], pT[:], AF.Sin)
                csT_c = scr.tile([128, NTC * 64], BF16, tag="csTc")
                nc.scalar.activation(csT_c[:], pT[:], AF.Sin, bias=HPI)

                # one accumulation chain at a time: a second open group in
                # the same psum bank zeroes the bank on this HW.
                def col(tensor, tau, a, b_):
                    w = 64 if tensor is csT_c or tensor is csT_s else 128
                    return tensor[:, tau * w + a:tau * w + b_]

                chains = [
                    [(csT_c, 0, 32, csT_c, 32, 64),
                     (csT_s, 0, 32, csT_s, 32, 64)],       # re
                    [(csT_s, 0, 32, csT_c, 32, 64)],       # im1
                    [(csT_c, 0, 32, csT_s, 32, 64)],       # im2
                    [(gpT, 0, 32, gpT, 32, 64)],           # tcorr
                    [(gpT, 64, 96, gpT, 96, 128)],         # pcorr
                ]
                for qi, terms in enumerate(chains):
                    g = ggp.tile([32, 32], F32, tag="g")
                    n = len(terms) * NTC
                    k = 0
                    for tau in range(NTC):
                        for (tl, a1, b1_, tr, a2, b2_) in terms:
                            nc.tensor.matmul(
                                g[:], lhsT=col(tl, tau, a1, b1_),
                                rhs=col(tr, tau, a2, b2_),
                                start=(k == 0), stop=(k == n - 1))
                            k += 1
                    eng = nc.vector if qi % 2 == 0 else nc.scalar
                    if qi % 2 == 0:
                        nc.vector.tensor_copy(
                            gram_sb[:, u * 160 + qi * 32:
                                    u * 160 + (qi + 1) * 32], g[:])
                    else:
                        nc.scalar.copy(
                            gram_sb[:, u * 160 + qi * 32:
                                    u * 160 + (qi + 1) * 32], g[:])

            # ------- software pipeline -------
            # fwd fc0-2 feed stations 0-5 (unit 0); start inv(0) before
            # fc3/4 so the phase chain launches ~10us earlier.
            for fc in (0, 1, 2):
                emit_fwd_fc(fc)
            for s in range(6):
                emit_station(s)
            emit_inv_and_folds(0)
            prefetch_tb((6, 7, 8), nc.gpsimd)
            for fc in (3, 4):
                emit_fwd_fc(fc)
            nc.gpsimd.dma_start(dst_sb[:], dstD[:])
            nc.gpsimd.dma_start(pwst_sb[64:128, :], pwstD[64:128, :])
            nc.gpsimd.dma_start(ident_sb[:], identD[:])
            for s in (6, 7, 8):
                emit_station(s)
            _fwdp.__exit__(None, None, None)
            _dpp = tc.tile_pool(name="dpp", bufs=3, space="PSUM")
            dpp = _dpp.__enter__()
            _ppp = tc.tile_pool(name="ppp", bufs=1, space="PSUM")
            ppp = _ppp.__enter__()
            _tpp = tc.tile_pool(name="tpp", bufs=1, space="PSUM")
            tpp = _tpp.__enter__()
            _ggp = tc.tile_pool(name="ggp", bufs=1, space="PSUM")
            ggp = _ggp.__enter__()
            emit_phases(0)
            for u in range(1, 3):
                emit_inv_and_folds(u)
                emit_pair(u - 1)
                emit_grams(u - 1)
                emit_phases(u)
            emit_pair(2)
            emit_grams(2)

            for cm in (_ggp, _tpp, _ppp, _dpp, psum_pools):
                cm.__exit__(None, None, None)

            # ------- outputs -------
            nc.sync.dma_start(o_aa[:], acc_a[:])
            nc.sync.dma_start(o_ac[:], acc_c[:])
            nc.sync.dma_start(o_aw[:], acc_w[:])
            nc.sync.dma_start(o_st[:], stats[:])
            nc.sync.dma_start(o_gram[:], gram_sb[:])

    legalize_waits(nc)
    return nc


_CACHE = {}


def _get_nc():
    if "nc" not in _CACHE:
        _CACHE["nc"] = build_nc()
    return _CACHE["nc"]


def _core_inputs(half, x64):
    if ("tab", half) not in _CACHE:
        _CACHE[("tab", half)] = host_tables(half)
    wtc, wts, tbl = _CACHE[("tab", half)]
    if "st" not in _CACHE:
        _CACHE["st"] = host_stations()
        _CACHE["ident"] = _bf16(np.eye(128, dtype=np.float32))
    dst, pwst = _CACHE["st"]
    xt = np.zeros((128, NTC * 64), np.float32)
    for tcb in range(NTC):
        xt[:, tcb * 64:(tcb + 1) * 64] = x64[:, tcb * 128:(tcb + 1) * 128].T
    return {
        "xt": _bf16(xt), "wtc": wtc, "wts": wts, "tbl": tbl,
        "dst": dst, "pwst": pwst, "ident": _CACHE["ident"],
    }


def _finalize(res_core, half):
    """res_core: per-core outputs -> {band: [7, C, C]}."""
    out = {}
    scale2 = (T / 2.0) ** 2
    st = np.asarray(res_core["o_st"], np.float64)
    for u, bi in enumerate(UNIT_BANDS[half]):
        g = np.asarray(res_core["o_gram"][:, u * 160:(u + 1) * 160],
                       np.float64)
        re, im1, im2 = g[:, 0:32], g[:, 32:64], g[:, 64:96]
        Gb, Gpw = g[:, 96:128], g[:, 128:160]
        plv = np.sqrt(re * re + (im1 - im2) ** 2) / T

        sc = u * 5
        Sb = st[0:64, sc] + st[64:128, sc]
        Sb2 = st[0:64, sc + 2] + st[64:128, sc + 2]
        Spw = st[0:64, sc + 3] + st[64:128, sc + 3]
        Spw2 = st[0:64, sc + 4] + st[64:128, sc + 4]

        def pair_sum(a, conv=None):
            cols = np.asarray(a[:, u * NPH:(u + 1) * NPH], np.float64)
            s = np.zeros(1024, np.float64)
            for ht in range(NPH):
                r = ht // 4
                v = cols[:, ht]
                if conv is not None:
                    v = conv(v, ht)
                s[r * 128:(r + 1) * 128] += v
            return s.reshape(32, 32)

        Sab = pair_sum(res_core["o_aa"])
        Ssgn = pair_sum(
            res_core["o_ac"],
            conv=lambda v, ht: 2.0 * v - TW
            if PASS_SCHED[ht // 4] == "S" else v)
        Sw = pair_sum(res_core["o_aw"])
        Spps = Spw[:C, None] + Spw[None, C:]          # sum_t (pw1_i + pw2_j)
        # S rows accumulate sum pw*[d>0] -> sgn-weighted = 2v - sum(pw);
        # P/Q rows accumulate sum pw*sgn directly.
        qmask = np.repeat(
            np.array([0.0 if PASS_SCHED[r] == "S" else 1.0
                      for r in range(NPT)]), 128).reshape(32, 32)
        Swp = qmask * Sw + (1.0 - qmask) * (2.0 * Sw - Spps)

        pli = np.abs(Ssgn) / T
        pdiff = Sab / T
        den = Spps + 2 * EPS * scale2
        wpli = np.abs(Swp) / den

        def corr2(G, S, S2):
            mu = S / T
            var = (S2 - T * mu * mu) / (T - 1)
            sd = np.sqrt(np.maximum(var, 0))
            N = G - T * np.outer(mu[:C], mu[C:])
            return N / ((np.outer(sd[:C], sd[C:]) + 1e-300) * T)

        tcorr = corr2(Gb, Sb, Sb2)
        pcorr = corr2(Gpw, Spw, Spw2)
        coh = np.full((C, C), len(band_rows(bi)) / NRFFT, np.float64)

        out[bi] = np.stack([plv, pli, wpli, coh, pcorr, pdiff,
                            tcorr]).astype(np.float32)
    return out


def kernel(eeg1, eeg2):
    eeg1 = np.asarray(eeg1, np.float32)
    eeg2 = np.asarray(eeg2, np.float32)
    B = eeg1.shape[0]
    nc = _get_nc()
    in_maps = []
    for c in range(8):
        b, half = c // 2, c % 2
        x64 = np.concatenate([eeg1[b], eeg2[b]], 0)
        in_maps.append(_core_inputs(half, x64))
    res = run_bass_kernel_spmd(nc, in_maps, core_ids=list(range(8)))
    out = np.zeros((B, 6, 7, C, C), np.float32)
    for c in range(8):
        b, half = c // 2, c % 2
        feats = _finalize(res.results[c], half)
        for bi, f in feats.items():
            out[b, bi] = f
    return out
